# revision 33
# baseline (speedup 1.0000x reference)
"""Cross-attention (GQA) Trainium2 Bass kernel — pipelined v3.

Problem: B=2, Tq=Tkv=2048, D_MODEL=1024, 16 query heads / 4 kv heads,
head_dim=64.  Sharded over 8 NeuronCores as batch(2) x kv-group(4); each
core computes 4 query heads + its single kv head and a partial output
projection (Wo row-split by head group); partials are summed on host.

Dataflow (feature dim on SBUF partitions end-to-end, no big transposes):

  A: qT[e,t] = WqT.T @ xqT,  kvT = WkvT.T @ xcT   (weights stationary)
     v[tk,dv] via PE-transpose of vT tiles; vp=[v|1], vp2=[1|v]
  B: per (sec=blk,e) unit t: pb[128,1024] = two K=64 row-group matmuls
     (h_even rows 0-63 -> cols 0:512, h_odd rows 64-127 -> cols
     512:1024), concurrent in the PE array.
  C: pt = exp(pb/8) one ScalarE instruction per unit (FD=1024).
  D: pd_h[128,512] += vp_t.T @ pt_half; ones-columns give the softmax
     denominators in the complementary 64 partitions.
  E: yT += WoT.T @ (pd*recip(den)), row-split by head pair.

v3 over v2:
  - ONE continuous 128-unit software pipeline (B(u), D(u-2), exp(u))
    across all 8 sections: no PE drain / exp restart bubble at section
    boundaries.
  - reciprocal_approx_fast (custom DVE op, ~5x faster than RECIPROCAL)
    in the softmax-denominator normalize chains.
  - Input DMAs split across BOTH hardware DGE queues: xq/wq/wo/ident
    issue from the (idle during lead-in) scalar/Act queue, xc/wkv from
    the sync queue, all unconditionally at kernel start so no waiting
    DMA ever sits in front of an input load.
  - Norm chains and output-projection (E) pieces pop from a dedicated
    "late" queue only at units t in [5..14] of the FOLLOWING section,
    when their DVE-side producers have long retired - they never stall
    the in-order PE.
  - y stores staged per tq-block in SBUF and written with 2 half-block
    DMAs (8 DMA issues/block -> 2), keeping the sync queue uncongested
    so the rec-broadcast DMAs on the norm critical path fire promptly.

ScalarE (the 1 elem/cycle/lane exp bottleneck, ~143us) paces the
kernel; the PE stays dense and HAM-warm.
"""

import os
import sys
from collections import deque

import numpy as np

for _p in ("/opt/trn_rl_repo",):
    if _p not in sys.path and os.path.isdir(_p):
        sys.path.insert(0, _p)

import concourse.bass as bass
import concourse.bacc as bacc
import concourse.mybir as mybir
from concourse.tile import TileContext

# ---------------------------------------------------------------- DVE exp ops
# Two custom DVE ops that together compute exp(x/8) = (1 + q)^32 with
# q = poly3(x/256) (Taylor-3 of e^{x/256}-1).  Offloading a few exp units
# per section to the DVE takes them off the ScalarE critical path (the
# ACT engine is the kernel's pacer).  Registered at import into the
# concourse dve_ops tables (name/opcode/spec), shas pinned from lower().
import concourse.dve_ops as _dve_ops
from concourse.dve_ops import DveOp as _DveOp
from concourse.dve_spec import (
    C0 as _C0, C1 as _C1, C2 as _C2, Spec as _Spec, Src0 as _Src0,
    _has_src1, lower as _lower, sq as _sq,
)
from concourse.dve_uop import DveOpSpec as _DveOpSpec

_S = 1.0 / 256.0
EXP_C0 = _S * _S * _S / 6.0
EXP_C1 = _S * _S / 2.0
EXP_C2 = _S


def _mk_dve_op(name, spec):
    if name in _dve_ops._SUB_OPCODE_FOR_NAME:
        return next(op for op in _dve_ops.OPS if op.name == name)
    opcode = _dve_ops._CUSTOM_DVE_ROW_BASE + len(_dve_ops.OPS)
    assert opcode < 0x20
    shas = {}
    for ver in ("v3", "v4"):
        try:
            s = _DveOpSpec(
                name=name, opcode=opcode, uops=_lower(spec, ver=ver),
                rd1_en=_has_src1(spec),
            )
            shas[ver] = s.sha(ver)
        except Exception:
            pass
    op = _DveOp(name, spec, False, shas)
    _dve_ops.OPS.append(op)
    _dve_ops.CUSTOM_DVE_SPECS[name] = spec
    _dve_ops._SUB_OPCODE_FOR_NAME[name] = opcode
    return op


EXP_POLY = _mk_dve_op(
    "EXP_POLY_ANT",
    _Spec(
        body=((_C0 * _Src0 + _C1) * _Src0 + _C2) * _Src0,
        reference=lambda in0, in1, c0, c1, c2: ((c0 * in0 + c1) * in0 + c2) * in0,
    ),
)
ONE_P_SQ5 = _mk_dve_op(
    "ONE_P_SQ5_ANT",
    _Spec(
        body=_sq(_sq(_sq(_sq(_sq(_Src0 + _C0))))),
        reference=lambda in0, in1, c0, c1, c2: (in0 + c0) ** 32,
    ),
)

# ---------------------------------------------------------------- problem dims
B = 2
TQ = 2048
TKV = 2048
D_MODEL = 1024
N_HEADS = 16
N_KV_HEADS = 4
HEAD_DIM = 64
N_CORES = 8
GROUPS = N_KV_HEADS  # kv groups = 4
HEADS_PER_DEV = N_HEADS // GROUPS  # 4
DQ = HEADS_PER_DEV * HEAD_DIM  # 256
DKV = 2 * HEAD_DIM  # 128 (k rows + v rows stacked)
SCALE = 1.0 / float(np.sqrt(HEAD_DIM))

P = 128
FREE = 512  # matmul moving-operand chunk / tq block width
BLK = 512
NBLK = TQ // BLK  # 4 tq blocks
DT = D_MODEL // P  # 8 d-tiles
ET = DQ // P  # 2 e-tiles (query head pairs)
NCH = TQ // FREE  # 4 x chunks of 512
NTK = TKV // P  # 16 tk tiles
MT = D_MODEL // P  # 8 output m-tiles
NSEC = NBLK * ET  # 8 sections
NU = NSEC * NTK  # 128 pipelined units

F32 = mybir.dt.float32
F16 = mybir.dt.float16


def build_bass():
    nc = bacc.Bacc()

    # all inputs/outputs are host-pre-arranged to be contiguous per SBUF
    # partition: each load/store is ~128 large descriptors, not 1024 small
    # ones (DGE issue cost and HBM efficiency both scale with that).
    xq = nc.declare_dram_parameter("xqh", [P, NCH, DT, FREE], F16, isOutput=False)
    xc = nc.declare_dram_parameter("xch", [P, NCH, DT, FREE], F16, isOutput=False)
    wq = nc.declare_dram_parameter("wqh", [P, DT, DQ], F16, isOutput=False)
    wkv = nc.declare_dram_parameter("wkvh", [P, DT, DKV], F16, isOutput=False)
    wo = nc.declare_dram_parameter("woh", [P, ET, D_MODEL], F16, isOutput=False)
    # block-swap matrix [[0,I64],[I64,0]]: cid2[64:, :64] is a plain I64 for
    # PE transposes; the full matrix PE-shifts kT from partitions 0-63 into
    # 64-127 (replacing a scheduler-hostile SBUF->SBUF broadcast DMA).
    cid = nc.declare_dram_parameter("cid2", [P, P], F16, isOutput=False)
    yt = nc.declare_dram_parameter("yh", [P, NBLK, MT, FREE], F16, isOutput=True)

    with TileContext(nc) as tc:
        with (
            tc.tile_pool(name="consts", bufs=1) as consts,
            tc.tile_pool(name="pt", bufs=6) as ptpool,
            tc.tile_pool(name="qx", bufs=2) as qxpool,
            tc.tile_pool(name="rec", bufs=2) as recpool,
            tc.tile_pool(name="yout", bufs=2) as ypool,
            tc.tile_pool(name="psS", bufs=2, space="PSUM") as psS,
            tc.tile_pool(name="psD", bufs=1, space="PSUM") as psD,
            tc.tile_pool(name="psA", bufs=2, space="PSUM") as psA,
        ):
            # ---------------- persistent tiles
            qt = consts.tile([P, ET, TQ], F16, tag="qt")  # head pair per e
            kv = consts.tile([P, TKV], F16, tag="kv")  # rows 0-63 kT, 64-127 vT
            k2 = consts.tile([P, TKV], F16, tag="k2")  # rows 64-127 = kT copy
            vp = consts.tile([P, NTK, P], F16, tag="vp")  # [v | ones]
            vp2 = consts.tile([P, NTK, P], F16, tag="vp2")  # [ones | v]
            outs = consts.tile([P, ET, TQ], F16, tag="outs")  # normalized outT
            ident = consts.tile([P, P], F16, tag="ident")
            wkv_sb = consts.tile([P, DT, DKV], F16, tag="wkv")
            wq_sb = consts.tile([P, DT, DQ], F16, tag="wq")
            wo_sb = consts.tile([P, ET, D_MODEL], F16, tag="wo")
            wrm = consts.tile([P, P], F16, tag="wrm")
            xq_t = [
                consts.tile([P, DT, FREE], F16, tag=f"xq{c}", name=f"xq{c}")
                for c in range(NCH)
            ]
            xc_t = [
                consts.tile([P, DT, FREE], F16, tag=f"xc{c}", name=f"xc{c}")
                for c in range(NCH)
            ]

            # ---------------- input DMA burst: everything issues up front,
            # split across the two hardware DGE queues.  Neither queue has a
            # waiting DMA in front of an input load.  Most-urgent first.
            # sync queue: kv path (xc chunks feed the B-matmul deadline
            # chain), then the non-urgent q-path chunks.
            # chunk 0 lands in quarters (one per projection piece) so the PE
            # ramps continuously instead of stalling on whole-chunk arrivals
            nc.sync.dma_start(wkv_sb, wkv[:])
            for qi in range(4):
                nc.sync.dma_start(
                    xc_t[0][:, 2 * qi : 2 * qi + 2, :], xc[:, 0, 2 * qi : 2 * qi + 2, :]
                )
            H = DT // 2
            nc.sync.dma_start(xc_t[1][:, :H, :], xc[:, 1, :H, :])
            nc.sync.dma_start(xc_t[1][:, H:, :], xc[:, 1, H:, :])
            for c in range(2, NCH):
                nc.sync.dma_start(xc_t[c], xc[:, c, :, :])
            for c in range(1, NCH):
                nc.sync.dma_start(xq_t[c], xq[:, c, :, :])
            nc.sync.dma_start(wo_sb, wo[:])
            # scalar/Act queue (idle until the first exp): only what the
            # first B matmul needs, so the issue cost never delays exp(0).
            # ident first: it gates the k2 shift and the v transposes.
            nc.scalar.dma_start(ident, cid[:])
            nc.scalar.dma_start(wq_sb, wq[:])
            for qi in range(4):
                nc.scalar.dma_start(
                    xq_t[0][:, 2 * qi : 2 * qi + 2, :], xq[:, 0, 2 * qi : 2 * qi + 2, :]
                )

            # vector-side const init + HAM warm-up (no DMA dependencies:
            # wrm memset feeds dummy matmuls that spin the PE clock up while
            # the inputs stream; a tiny exp pulls the ACT table load early).
            nc.vector.memset(wrm, 0.125)
            nc.vector.memset(vp, 1.0)
            nc.vector.memset(vp2, 1.0)
            dum = consts.tile([P, 8], F16, tag="dum")
            nc.scalar.activation(
                dum, wrm[:, :8], mybir.ActivationFunctionType.Exp, bias=0.0, scale=1.0
            )
            warm = psA.tile([P, P], F32, tag="pa", name="warm")
            for i in range(20):
                nc.tensor.matmul(warm, wrm, wrm, start=(i == 0), stop=(i == 19))

            # ---------------- fill-work machinery (PE slack consumers)
            fills = deque()  # anytime work: projections, transposes, dma issues
            late = deque()  # dep-settled-late work: norm chains, E pieces

            def pop_fill(n=1):
                for _ in range(n):
                    if not fills:
                        return
                    fills.popleft()()

            # D matmuls for one pipelined unit (two heads, K=128, N=512)
            def emit_d(pd0, pd1, pt, t):
                nc.tensor.matmul(
                    pd0, vp[:, t, :], pt[:, :BLK],
                    start=(t == 0), stop=(t == NTK - 1), skip_group_check=True,
                )
                nc.tensor.matmul(
                    pd1, vp2[:, t, :], pt[:, BLK:],
                    start=(t == 0), stop=(t == NTK - 1), skip_group_check=True,
                )

            # kv projection chunk: 8 K-tiles -> kv[:, cs]; k2 shift; transposes
            def kv_chunk_pieces(c):
                cs = slice(c * FREE, (c + 1) * FREE)
                st = {}

                def pk(i0):
                    def p():
                        if i0 == 0:
                            st["pkv"] = psA.tile([P, FREE], F32, tag="pa", name="pkv")
                        for i in range(i0, i0 + 2):
                            nc.tensor.matmul(
                                st["pkv"], wkv_sb[:, i, :], xc_t[c][:, i, :],
                                start=(i == 0), stop=(i == DT - 1),
                            )
                        if i0 == DT - 2:
                            nc.vector.tensor_copy(kv[:, cs], st["pkv"])

                    return p

                def p4():
                    # kT -> partitions 64-127 of k2 via the block-swap matmul
                    psK = psA.tile([P, FREE], F32, tag="pa", name="psK")
                    nc.tensor.matmul(psK, ident[:HEAD_DIM, :], kv[:HEAD_DIM, cs])
                    nc.vector.tensor_copy(k2[HEAD_DIM:, cs], psK[HEAD_DIM:, :])

                def p3():
                    # transpose the 4 v tiles of this chunk, batch-copy to vp/vp2
                    pvb = psA.tile([P, 4 * HEAD_DIM], F16, tag="pa", name="pvb")
                    for k in range(4):
                        ts_ = slice((4 * c + k) * P, (4 * c + k + 1) * P)
                        nc.tensor.transpose(
                            pvb[:, k * HEAD_DIM : (k + 1) * HEAD_DIM],
                            kv[HEAD_DIM:, ts_],
                            ident[HEAD_DIM:, :HEAD_DIM],
                        )
                    src = pvb.rearrange("p (k d) -> p k d", k=4)
                    nc.vector.tensor_copy(vp[:, 4 * c : 4 * c + 4, :HEAD_DIM], src)
                    nc.vector.tensor_copy(vp2[:, 4 * c : 4 * c + 4, HEAD_DIM:], src)

                return [pk(0), pk(2), pk(4), pk(6), p4, p3]

            # q projection chunk (one e): 8 K-tiles -> qt[:, e, cs]
            def q_chunk_pieces(c, e):
                cs = slice(c * FREE, (c + 1) * FREE)
                st = {}

                def pq(i0):
                    def p():
                        if i0 == 0:
                            st["pq"] = psA.tile([P, FREE], F32, tag="pa", name="pq")
                        for i in range(i0, i0 + 2):
                            nc.tensor.matmul(
                                st["pq"], wq_sb[:, i, e * P : (e + 1) * P],
                                xq_t[c][:, i, :],
                                start=(i == 0), stop=(i == DT - 1),
                            )
                        if i0 == DT - 2:
                            nc.vector.tensor_copy(qt[:, e, cs], st["pq"])

                    return p

                return [pq(0), pq(2), pq(4), pq(6)]

            # output projection for one tq block: 8 m-tile pieces staged into
            # ySB, then half-block DMA issues.  In tail mode the py PSUM
            # accumulators rotate over 4 banks (psA pair + the pd banks,
            # which are free once the final spill ran) so the E matmuls
            # never stall on the yo casts, and stores are finer-grained so
            # the last transfer is small.
            def e_pieces(blk, tail=False):
                bs = slice(blk * BLK, (blk + 1) * BLK)
                ysb = ypool.tile([P, MT, FREE], F16, tag="ysb", name=f"ysb{blk}")

                def mk(m):
                    def p():
                        if tail and m % 2 == 1:
                            tag = "pd0" if m % 4 == 1 else "pd1"
                            py = psD.tile([P, FREE], F32, tag=tag, name="py")
                        else:
                            py = psA.tile([P, FREE], F32, tag="pa", name="py")
                        for ee in range(ET):
                            nc.tensor.matmul(
                                py, wo_sb[:, ee, m * P : (m + 1) * P], outs[:, ee, bs],
                                start=(ee == 0), stop=(ee == ET - 1),
                            )
                        nc.vector.tensor_copy(ysb[:, m, :], py)

                    return p

                def store(h, nst):
                    def p():
                        ms = slice(h * (MT // nst), (h + 1) * (MT // nst))
                        nc.sync.dma_start(yt[:, blk, ms, :], ysb[:, ms, :])

                    return p

                pieces = [mk(m) for m in range(MT)]
                if tail:
                    out = []
                    for m in range(MT):
                        out.append(pieces[m])
                        if m % 2 == 1:
                            out.append(store(m // 2, 4))
                    return out
                return pieces + [store(0, 2), store(1, 2)]

            # spill pd0/pd1 for one section into aligned full-partition
            # tiles: rawN = [AV_even | AV_odd], rawD = [den_odd | den_even]
            # (reciprocal_approx_fast silently corrupts partition-offset
            # operands, so the custom op must see full offset-0 tiles).
            def spill(dp0, dp1):
                rawN = recpool.tile([P, BLK], F32, tag="rawN", name="rawN")
                rawD = recpool.tile([P, BLK], F32, tag="rawD", name="rawD")
                nc.vector.tensor_copy(rawN[:HEAD_DIM, :], dp0[:HEAD_DIM, :])
                nc.vector.tensor_copy(rawN[HEAD_DIM:, :], dp1[HEAD_DIM:, :])
                nc.vector.tensor_copy(rawD[:HEAD_DIM, :], dp1[:HEAD_DIM, :])
                nc.vector.tensor_copy(rawD[HEAD_DIM:, :], dp0[HEAD_DIM:, :])
                return rawN, rawD

            # normalize chain for one section (deferred into the late window
            # of the following section): one full-partition fast recip, two
            # half-swap broadcast DMAs, one full-width multiply.
            def norm_chain(sec, rawN, rawD):
                blk, e = divmod(sec, ET)
                bs = slice(blk * BLK, (blk + 1) * BLK)

                def p():
                    recD = recpool.tile([P, BLK], F32, tag="recD", name="recD")
                    recS = recpool.tile([P, BLK], F32, tag="recS", name="recS")
                    nc.vector.reciprocal_approx_fast(recD, rawD)
                    nc.sync.dma_start(recS[:HEAD_DIM, :], recD[HEAD_DIM:, :])
                    nc.sync.dma_start(recS[HEAD_DIM:, :], recD[:HEAD_DIM, :])
                    nc.vector.tensor_mul(outs[:, e, bs], rawN, recS)

                return p

            # ---------------- lead-in PE work: kv chunk 0 + q chunk 0 (e=0)
            # inline; everything else is fills with deadline-ordered layout.
            # interleaved so the PE consumes each quarter-chunk DMA as it lands
            kc0 = kv_chunk_pieces(0)
            q00 = q_chunk_pieces(0, 0)
            kc0[0]()
            q00[0]()
            kc0[1]()
            q00[1]()
            kc0[2]()
            q00[2]()
            kc0[3]()
            q00[3]()
            kc0[4]()  # k2 shift for chunk 0 (B(0) reads it)

            # Fill deadline order for section 0 (2 pops/unit):
            #   kc0.p3 (v transposes, before D(0) at u=2) -> t=0
            #   kc{c}: pk x4 + p4 (k2 shift) before B(4c); p3 before D(4c).
            fills.append(kc0[5])
            fills.extend(kv_chunk_pieces(1))
            fills.extend(kv_chunk_pieces(2))
            fills.extend(kv_chunk_pieces(3))
            fills.extend(q_chunk_pieces(0, 1))
            for e in range(ET):
                fills.extend(q_chunk_pieces(1, e))
            for e in range(ET):
                fills.extend(q_chunk_pieces(2, e))
            for e in range(ET):
                fills.extend(q_chunk_pieces(3, e))

            # ---------------- the continuous BCD pipeline over 128 units
            units = [(sec, t) for sec in range(NSEC) for t in range(NTK)]
            pending = deque()  # (pd0, pd1, pt, t, sec, u, offloaded)
            pd_cur = None

            def drain_ready(u):
                # D-lag is 2 for ScalarE exp units but 6 for DVE-offloaded
                # ones (the in-order DVE queue delivers their pt later).
                while pending:
                    _, _, _, _, _, pu, off = pending[0]
                    if u - pu < (6 if off else 2):
                        return
                    dp0, dp1, dpt, dt_, dsec, _, _ = pending.popleft()
                    emit_d(dp0, dp1, dpt, dt_)
                    if dt_ == NTK - 1 and dsec < NSEC - 1:
                        # section dsec fully accumulated: spill pd -> raw
                        # (frees the PSUM banks for this section's own Ds),
                        # queue the normalize + block-complete E work.
                        rawN, rawD = spill(dp0, dp1)
                        late.append(norm_chain(dsec, rawN, rawD))
                        if dsec % ET == ET - 1 and dsec >= 1:
                            late.extend(e_pieces(dsec // ET))

            for u, (sec, t) in enumerate(units):
                blk, e = divmod(sec, ET)
                bs = slice(blk * BLK, (blk + 1) * BLK)
                # exp units handed to the DVE custom ops: placed late in the
                # section where the DVE queue (spills/norm/casts) has
                # drained, so pb release never stalls the B stream.  The
                # final section keeps t=15 on ScalarE for a clean drain.
                if sec == NSEC - 1:
                    off_t = (12,)
                elif sec % 2 == 1:
                    off_t = (12, 15)
                else:
                    off_t = (15,)
                if t == 0:
                    pd_cur = (
                        psD.tile([P, BLK], F32, tag="pd0", name="pd0"),
                        psD.tile([P, BLK], F32, tag="pd1", name="pd1"),
                    )
                pb = psS.tile([P, 2 * BLK], F32, tag="pb", name="pb")
                # B: two K=64 row-group matmuls, concurrent in the array
                nc.tensor.matmul(pb[:, :BLK], kv[:HEAD_DIM, t * P : (t + 1) * P],
                                 qt[:HEAD_DIM, e, bs])
                nc.tensor.matmul(pb[:, BLK:], k2[HEAD_DIM:, t * P : (t + 1) * P],
                                 qt[HEAD_DIM:, e, bs])
                drain_ready(u)
                pt = ptpool.tile([P, 2 * BLK], F16, tag="pt", name="pt")
                offloaded = t in off_t
                if offloaded:
                    qx = qxpool.tile([P, 2 * BLK], F32, tag="qx", name="qx")
                    nc.vector._custom_dve(
                        EXP_POLY, out=qx, in0=pb, s0=EXP_C0, s1=EXP_C1, imm2=EXP_C2
                    )
                    nc.vector._custom_dve(ONE_P_SQ5, out=pt, in0=qx, s0=1.0)
                else:
                    nc.scalar.activation(
                        pt, pb, mybir.ActivationFunctionType.Exp, bias=0.0, scale=SCALE
                    )
                pending.append((pd_cur[0], pd_cur[1], pt, t, sec, u, offloaded))
                if sec == 0:
                    # 2/unit covers the kv deadline chain + q(0,1); beyond
                    # that, spread the q projections over later sections so
                    # the half-clocked lead-in PE isn't oversubscribed.
                    pop_fill(2 if t < 12 else 1)
                elif 5 <= t <= 14 and late:
                    late.popleft()()
                elif t >= 2:
                    pop_fill(1)

            # ---------------- tail: drain last two Ds, normalize the final
            # section straight out of PSUM (fast recip), output-project the
            # last block, store.
            final_pd = None
            while pending:
                dp0, dp1, dpt, dt_, dsec, _, _ = pending.popleft()
                emit_d(dp0, dp1, dpt, dt_)
                final_pd = (dp0, dp1)
            # warm-keepers: run during the ~3us norm chain below so the PE
            # P-state stays at full clock for the output projection.
            wt = psA.tile([P, P], F32, tag="pa", name="wt")
            for i in range(24):
                nc.tensor.matmul(wt, wrm, wrm, start=(i == 0), stop=(i == 23))
            while late:
                late.popleft()()
            # tail normalize, minimum latency: den spill only, fast recip,
            # fp16 cast + PE block-swap (no DMA round trip), muls straight
            # from the pd PSUM banks.
            dp0, dp1 = final_pd
            e, bs = 1, slice((NBLK - 1) * BLK, NBLK * BLK)
            rawD = recpool.tile([P, BLK], F32, tag="rawD", name="rawD")
            nc.vector.tensor_copy(rawD[:HEAD_DIM, :], dp1[:HEAD_DIM, :])
            nc.vector.tensor_copy(rawD[HEAD_DIM:, :], dp0[HEAD_DIM:, :])
            recD = recpool.tile([P, BLK], F32, tag="recD", name="recD")
            nc.vector.reciprocal_approx_fast(recD, rawD)
            recH = recpool.tile([P, BLK], F16, tag="recS", name="recH")
            nc.vector.tensor_copy(recH, recD)
            psR = psA.tile([P, BLK], F32, tag="pa", name="psR")
            nc.tensor.matmul(psR, ident, recH)
            recS = recpool.tile([P, BLK], F32, tag="rawN", name="recSf")
            nc.vector.tensor_copy(recS, psR)
            nc.vector.tensor_mul(outs[:HEAD_DIM, e, bs], dp0[:HEAD_DIM, :],
                                 recS[:HEAD_DIM, :])
            nc.vector.tensor_mul(outs[HEAD_DIM:, e, bs], dp1[HEAD_DIM:, :],
                                 recS[HEAD_DIM:, :])
            for piece in e_pieces(NBLK - 1, tail=True):
                piece()
            while fills:
                pop_fill()

    nc.finalize()
    return nc


_NC_CACHE = None


def _get_nc():
    global _NC_CACHE
    if _NC_CACHE is None:
        _NC_CACHE = build_bass()
    return _NC_CACHE


def _cid2():
    z = np.zeros((HEAD_DIM, HEAD_DIM), dtype=np.float16)
    i = np.eye(HEAD_DIM, dtype=np.float16)
    return np.block([[z, i], [i, z]])


def _chunked(xT):
    """[D_MODEL, T] -> [P, NCH, DT, FREE] with row i*P+p at [p, :, i, :]:
    each partition's chunk data contiguous for large-descriptor DMA."""
    return np.ascontiguousarray(
        xT.reshape(DT, P, NCH, FREE).transpose(1, 2, 0, 3)
    ).astype(np.float16)


def _wtiles(wT):
    """[D_MODEL, E] -> [P, DT, E]"""
    return np.ascontiguousarray(
        wT.reshape(DT, P, wT.shape[1]).transpose(1, 0, 2)
    ).astype(np.float16)


def shard_inputs(query, context, Wq, Wk, Wv, Wo):
    """host-side sharding: 8 cores = batch(2) x kv-group(4)"""
    in_maps = []
    xqh = [_chunked(np.asarray(query[b]).T) for b in range(B)]
    xch = [_chunked(np.asarray(context[b]).T) for b in range(B)]
    for core in range(N_CORES):
        b, g = divmod(core, GROUPS)
        wqh = _wtiles(Wq[g * DQ : (g + 1) * DQ, :].T)
        wkvh = _wtiles(
            np.concatenate(
                [
                    Wk[g * HEAD_DIM : (g + 1) * HEAD_DIM, :],
                    Wv[g * HEAD_DIM : (g + 1) * HEAD_DIM, :],
                ],
                axis=0,
            ).T
        )
        woT = Wo[:, g * DQ : (g + 1) * DQ].T  # [DQ, D_MODEL]
        woh = np.ascontiguousarray(
            woT.reshape(ET, P, D_MODEL).transpose(1, 0, 2)
        ).astype(np.float16)
        in_maps.append(
            {
                "xqh": xqh[b],
                "xch": xch[b],
                "wqh": wqh,
                "wkvh": wkvh,
                "woh": woh,
                "cid2": _cid2(),
            }
        )
    return in_maps


def kernel(query, context, Wq, Wk, Wv, Wo, _want_profile=False):
    from concourse.bass_utils import run_bass_kernel_spmd

    nc = _get_nc()
    in_maps = shard_inputs(query, context, Wq, Wk, Wv, Wo)
    res = run_bass_kernel_spmd(
        nc, in_maps, core_ids=list(range(N_CORES)), trace=_want_profile
    )
    out = np.zeros((B, TQ, D_MODEL), dtype=np.float32)
    for core in range(N_CORES):
        b = core // GROUPS
        yh = res.results[core]["yh"].astype(np.float32)
        yT = yh.transpose(2, 0, 1, 3).reshape(D_MODEL, TQ)
        out[b] += yT.T
    if _want_profile:
        return out, res
    return out
